# revision 1
# baseline (speedup 1.0000x reference)
"""Trainium2 Bass kernel for CoordLSVotingWeighted (segment_reduce).

Strategy: data-parallel over batch B=8 across 8 NeuronCores (1 image/core).
Per image, on device:
  - hard one-hot of argmax over 9 seg channels (matches softmax(seg*1e6))
  - unit-direction projection matrices R = w*(I - n n^T), w = softplus(w)
  - segment-reduce per class via TensorE matmul:
      psum[24,27] = sum_pix lhsT[pix, {hot, hot*ch, hot*cw}]^T
                    @ rhs[pix, {R00, m, R11}]   (m = -R01 = w*nx*ny/s)
Host: assemble 2x2 systems in float64, pinv-solve, scale by HEIGHT.

Self-contained: only needs numpy / ml_dtypes / concourse (installed env).
"""

import os

import numpy as np

B = 8
H = 128
W = 128
NCLS = 9  # seg channels, class 0 = background
NPTS = 9
OC = 8
HEIGHT = 128.0
N_CORES = 8

_cache: dict = {}


def _build_nc():
    import concourse.bacc as bacc
    import concourse.tile as tile
    import concourse.mybir as mybir
    from concourse.alu_op_type import AluOpType as Alu

    Act = mybir.ActivationFunctionType
    Axis = mybir.AxisListType
    f32 = mybir.dt.float32
    b16 = mybir.dt.bfloat16

    nc = bacc.Bacc(
        "TRN2", target_bir_lowering=False, debug=False, num_devices=N_CORES
    )
    seg_d = nc.dram_tensor("seg", [H, W * NCLS], f32, kind="ExternalInput")
    dct_d = nc.dram_tensor("direct", [H, W * NPTS * 2], f32, kind="ExternalInput")
    w_d = nc.dram_tensor("w", [H, W * NPTS], f32, kind="ExternalInput")
    cw_d = nc.dram_tensor("cw8", [H, OC * W], b16, kind="ExternalInput")
    ch_d = nc.dram_tensor("chv", [H, 1], f32, kind="ExternalInput")
    out_d = nc.dram_tensor("acc", [3 * OC, 3 * NPTS], f32, kind="ExternalOutput")

    NF = W * NPTS  # 1152

    with tile.TileContext(nc) as tc:
        with (
            tc.tile_pool(name="main", bufs=1) as pool,
            tc.tile_pool(name="ps", bufs=1, space="PSUM") as psp,
        ):
            # ---- input tiles
            sgt = pool.tile([H, W * NCLS], f32, tag="sgt")
            dct = pool.tile([H, W * NPTS * 2], f32, tag="dct")
            wdt = pool.tile([H, W * NPTS], f32, tag="wdt")
            cwt = pool.tile([H, OC * W], b16, tag="cwt")
            cht = pool.tile([H, 1], f32, tag="cht")
            # two DMA queues in parallel: {w, seg, ch} on sync, {direct, cw} on vector
            nc.sync.dma_start(out=wdt[:, :], in_=w_d[:, :])
            nc.sync.dma_start(out=dct[:, :], in_=dct_d[:, :])
            nc.sync.dma_start(out=sgt[:, :], in_=seg_d[:, :])
            nc.sync.dma_start(out=cwt[:, :], in_=cw_d[:, :])
            nc.sync.dma_start(out=cht[:, :], in_=ch_d[:, :])

            # ---- work tiles (bf16 unless noted)
            sq = pool.tile([H, 2 * NF], b16, tag="sq")     # [x^2|y^2]
            s16 = pool.tile([H, NF], b16, tag="s16")
            ls32 = pool.tile([H, NF], f32, tag="ls32")
            rr16 = pool.tile([H, NF], b16, tag="rr16")
            ew16 = pool.tile([H, NF], b16, tag="ew16")
            sp16 = pool.tile([H, NF], b16, tag="sp16")
            k16 = pool.tile([H, NF], b16, tag="k16")
            u16 = pool.tile([H, NF], b16, tag="u16")
            mx = pool.tile([H, W], f32, tag="mx")
            b9 = pool.tile([H, 1], f32, tag="b9")
            nc.vector.memset(b9[:, :], 1e-9)
            L = pool.tile([H, 3 * OC * W], b16, tag="L")   # hot|hot*ch|hot*cw
            R = pool.tile([H, 3 * NF], b16, tag="R")       # R00|m|R11
            outs = pool.tile([3 * OC, 3 * NPTS], f32, tag="outs")

            # ---- one-hot lhs first: depends only on seg DMA
            sgt_wc = sgt[:, :].rearrange("q (w c) -> q w c", c=NCLS)
            nc.vector.tensor_reduce(
                out=mx[:, :], in_=sgt_wc, axis=Axis.X, op=Alu.max
            )
            sgt_cw = sgt[:, :].rearrange("q (w c) -> q c w", c=NCLS)[:, 1:NCLS, :]
            mx_b = mx[:, :].unsqueeze(1).broadcast_to((H, OC, W))
            hot_r = L[:, 0 : OC * W].rearrange("q (c w) -> q c w", c=OC)
            nc.vector.tensor_tensor(
                out=hot_r, in0=sgt_cw, in1=mx_b, op=Alu.is_equal
            )
            nc.vector.tensor_scalar_mul(
                L[:, OC * W : 2 * OC * W], L[:, 0 : OC * W], cht[:, :]
            )
            nc.vector.tensor_tensor(
                out=L[:, 2 * OC * W : 3 * OC * W], in0=L[:, 0 : OC * W],
                in1=cwt[:, :], op=Alu.mult,
            )

            # ---- strided views of direct: nx = even cols, ny = odd cols
            nxs = dct[:, 0::2].rearrange("q (w g) -> q g w", g=NPTS)
            nys = dct[:, 1::2].rearrange("q (w g) -> q g w", g=NPTS)

            # ---- softplus(w) = Ln(Exp(w) + 1), single ACT table set (ln/exp)
            w_r = wdt[:, :].rearrange("q (w g) -> q g w", g=NPTS)
            ew_r = ew16[:, :].rearrange("q (g w) -> q g w", g=NPTS)
            nc.scalar.activation(out=ew_r, in_=w_r, func=Act.Exp)
            nc.scalar.activation(out=sp16[:, :], in_=ew16[:, :], func=Act.Ln, bias=1.0)

            # ---- squares via ACT (Square is in the resident table set)
            sqx_r = sq[:, 0:NF].rearrange("q (g w) -> q g w", g=NPTS)
            sqy_r = sq[:, NF : 2 * NF].rearrange("q (g w) -> q g w", g=NPTS)
            nc.vector.tensor_tensor(out=sqx_r, in0=nxs, in1=nxs, op=Alu.mult)
            nc.vector.tensor_tensor(out=sqy_r, in0=nys, in1=nys, op=Alu.mult)
            nc.vector.tensor_tensor(
                out=s16[:, :], in0=sq[:, 0:NF], in1=sq[:, NF : 2 * NF], op=Alu.add
            )
            nc.scalar.activation(
                out=ls32[:, :], in_=s16[:, :], func=Act.Ln, bias=b9[:, :]
            )
            nc.scalar.activation(out=rr16[:, :], in_=ls32[:, :], func=Act.Exp, scale=-1.0)

            # ---- k = softplus(w)/s ; rhs features R00=k*ny^2, m=k*nx*ny, R11=k*nx^2
            nc.vector.tensor_tensor(
                out=k16[:, :], in0=sp16[:, :], in1=rr16[:, :], op=Alu.mult
            )
            nc.vector.tensor_tensor(
                out=R[:, 0:NF], in0=k16[:, :], in1=sq[:, NF : 2 * NF], op=Alu.mult
            )
            k16_r = k16[:, :].rearrange("q (g w) -> q g w", g=NPTS)
            u16_r = u16[:, :].rearrange("q (g w) -> q g w", g=NPTS)
            nc.vector.tensor_tensor(out=u16_r, in0=k16_r, in1=nxs, op=Alu.mult)
            nc.vector.tensor_tensor(
                out=R[:, NF : 2 * NF].rearrange("q (g w) -> q g w", g=NPTS),
                in0=u16_r, in1=nys, op=Alu.mult,
            )
            nc.vector.tensor_tensor(
                out=R[:, 2 * NF : 3 * NF], in0=k16[:, :], in1=sq[:, 0:NF], op=Alu.mult
            )

            # ---- segment reduce: 128 accumulating matmuls over w-chunks
            acc = psp.tile([3 * OC, 3 * NPTS], f32, tag="acc")
            for wi in range(W):
                nc.tensor.matmul(
                    acc[:, :],
                    L[:, wi :: W],
                    R[:, wi :: W],
                    start=(wi == 0),
                    stop=(wi == W - 1),
                )

            nc.vector.tensor_copy(out=outs[:, :], in_=acc[:, :])
            nc.sync.dma_start(out=out_d[:, :], in_=outs[:, :])

    nc.compile()
    return nc


def _host_constants():
    import ml_dtypes

    bf16 = ml_dtypes.bfloat16
    coord = ((np.arange(128, dtype=np.float32) + 0.5) / HEIGHT).astype(bf16)
    cw8 = np.ascontiguousarray(
        np.broadcast_to(coord[None, None, :], (H, OC, W))
    ).reshape(H, OC * W)
    chv = ((np.arange(128, dtype=np.float32) + 0.5) / HEIGHT).reshape(H, 1)
    return cw8, chv


def _solve_host(acc_f32: np.ndarray) -> np.ndarray:
    """acc [24,27] fp32 -> p [OC, NPTS, 2] fp32 (float64 pinv like reference)."""
    a = acc_f32.astype(np.float64)
    A = a[0:OC, 0:9]
    Bm = a[0:OC, 9:18]
    D = a[0:OC, 18:27]
    S1 = a[OC : 2 * OC, 0:9]
    S3 = a[OC : 2 * OC, 9:18]
    S2 = a[2 * OC : 3 * OC, 9:18]
    S4 = a[2 * OC : 3 * OC, 18:27]
    Rm = np.empty((OC, NPTS, 2, 2), dtype=np.float64)
    Rm[..., 0, 0] = A
    Rm[..., 0, 1] = -Bm
    Rm[..., 1, 0] = -Bm
    Rm[..., 1, 1] = D
    q = np.stack([S1 - S2, S4 - S3], axis=-1)
    Rp = np.linalg.pinv(Rm.reshape(-1, 2, 2)).reshape(Rm.shape)
    p = np.einsum("cpij,cpj->cpi", Rp, q) * HEIGHT
    return p.astype(np.float32)


def kernel(seg, direct, w):
    if "nc" not in _cache:
        _cache["nc"] = _build_nc()
    nc = _cache["nc"]

    seg = np.ascontiguousarray(np.asarray(seg, dtype=np.float32))
    direct = np.ascontiguousarray(np.asarray(direct, dtype=np.float32))
    w = np.ascontiguousarray(np.asarray(w, dtype=np.float32))
    cw8, chv = _host_constants()

    in_maps = []
    for i in range(B):
        in_maps.append(
            {
                "seg": seg[i].reshape(H, W * NCLS),
                "direct": direct[i].reshape(H, W * NPTS * 2),
                "w": w[i].reshape(H, W * NPTS),
                "cw8": cw8,
                "chv": chv,
            }
        )

    from concourse.bass_utils import run_bass_kernel_spmd

    trace = bool(int(os.environ.get("KERNEL_TRACE", "0")))
    res = run_bass_kernel_spmd(
        nc, in_maps, core_ids=list(range(N_CORES)), trace=trace
    )
    kernel._last_exec_ns = res.exec_time_ns
    kernel._last_results = res

    out = np.stack(
        [_solve_host(np.asarray(res.results[i]["acc"])) for i in range(B)], axis=0
    )
    return out



# revision 6
# speedup vs baseline: 1.3606x; 1.3606x over previous
"""Trainium2 Bass kernel for CoordLSVotingWeighted (segment_reduce).

Strategy: data-parallel over batch B=8 across 8 NeuronCores (1 image/core).
Per image, on device (pipelined over 4 sub-chunks = 2 H-halves x 2 W-slices):
  - hard one-hot of argmax over 9 seg channels (matches softmax(seg*1e6))
  - features R00 = u*y^2, m = u*x*y, R11 = u*x^2 with u = softplus(w)/(x^2+y^2)
  - segment-reduce per class via TensorE: psum[24,27] accumulates
      lhsT[pix, {hot, hot*ch, hot*cw}]^T @ rhs[pix, {R00, m, R11}]
    over 128 pixel-group matmuls.
Host: assemble 2x2 systems in float64, pinv-solve, scale by HEIGHT.

Layout: an H-half (64 rows) of each input is a single contiguous DRAM block
loaded as [128 partitions, cols]: partition = 2*(h%64) + w//64. Engine split:
ACT does squares/softplus/hot*ch (single act table: exp+ln+square+copy),
DVE does sums/reciprocal/feature mults/hot*cw, Pool does max/one-hot/xy.

Self-contained: only needs numpy / ml_dtypes / concourse (installed env).
"""

import os

import numpy as np

B = 8
H = 128
W = 128
NCLS = 9  # seg channels, class 0 = background
NPTS = 9
OC = 8
HEIGHT = 128.0
N_CORES = 8

_cache: dict = {}


def _patch_act_tables():
    """Make {Exp, Ln, Square, Copy, Identity} resolve only to the
    natural_log_exp_and_others table so the table-load pass emits exactly
    one ACT_TABLE_LOAD (baseline thrashed 3 loads x 1.28us). Keeps every
    table's canonical index so emitted act_func_set_ids stay valid."""
    import concourse.bacc as bacc_mod
    import concourse.mybir as mybir

    if _cache.get("act_patched"):
        return
    real = bacc_mod.get_activation_tables
    F = mybir.ActivationFunctionType
    shared = {F.Exp, F.Ln, F.Square, F.Copy, F.Identity}

    def patched(arch):
        tables = real(arch)
        out = {}
        for name, funcs in tables.items():
            if name == "natural_log_exp_and_others":
                out[name] = funcs
            else:
                out[name] = funcs - shared
        return out

    bacc_mod.get_activation_tables = patched
    _cache["act_patched"] = True


def _build_nc():
    _patch_act_tables()
    import concourse.bacc as bacc
    import concourse.tile as tile
    import concourse.mybir as mybir
    from concourse.alu_op_type import AluOpType as Alu

    Act = mybir.ActivationFunctionType
    Axis = mybir.AxisListType
    f32 = mybir.dt.float32
    b16 = mybir.dt.bfloat16

    nc = bacc.Bacc(
        "TRN2", target_bir_lowering=False, debug=False, num_devices=N_CORES
    )
    # One image per core; an H-half (64 rows) is contiguous in DRAM and maps
    # to 128 SBUF partitions: dram row r = 2*h + w//64  (r in [0,256)).
    seg_d = nc.dram_tensor("seg", [2 * H, W * NCLS // 2], f32, kind="ExternalInput")
    dct_d = nc.dram_tensor("direct", [2 * H, W * NPTS], f32, kind="ExternalInput")
    w_d = nc.dram_tensor("w", [2 * H, W * NPTS // 2], f32, kind="ExternalInput")
    chv_d = nc.dram_tensor("chv", [H, 2], f32, kind="ExternalInput")
    cw_d = nc.dram_tensor("cw64", [H, 64], b16, kind="ExternalInput")
    out_d = nc.dram_tensor("acc", [3 * OC, 3 * NPTS], f32, kind="ExternalOutput")

    SEGC = 576   # seg cols per half-tile   (64 w * 9 c)
    DCTC = 1152  # direct cols per half-tile (64 w * 9 g * 2)
    WC = 576     # w cols per half-tile      (64 w * 9 g)
    CH = 288     # feature cols per sub-chunk (32 w * 9 g)
    HC = 256     # one-hot cols per sub-chunk (8 c * 32 w)

    with tile.TileContext(nc) as tc:
        with (
            tc.tile_pool(name="main", bufs=1) as pool,
            tc.tile_pool(name="ps", bufs=1, space="PSUM") as psp,
        ):
            chv = pool.tile([H, 2], f32, tag="chv")
            cw64 = pool.tile([H, 64], b16, tag="cw64")
            seg_h = [pool.tile([H, SEGC], f32, name=f"seg{h}", tag=f"seg{h}") for h in range(2)]
            dct_h = [pool.tile([H, DCTC], f32, name=f"dct{h}", tag=f"dct{h}") for h in range(2)]
            w_h = [pool.tile([H, WC], f32, name=f"w{h}", tag=f"w{h}") for h in range(2)]

            # DMA: consts + {w, seg} halves + direct1 on sync queue;
            # direct0 (needed early by ACT squares) on gpsimd queue.
            nc.sync.dma_start(out=chv[:, :], in_=chv_d[:, :])
            nc.sync.dma_start(out=cw64[:, :], in_=cw_d[:, :])
            nc.gpsimd.dma_start(out=dct_h[0][:, :], in_=dct_d[0:128, :])
            nc.sync.dma_start(out=w_h[0][:, :], in_=w_d[0:128, :])
            nc.sync.dma_start(out=seg_h[0][:, :], in_=seg_d[0:128, :])
            nc.sync.dma_start(out=w_h[1][:, :], in_=w_d[128:256, :])
            nc.sync.dma_start(out=seg_h[1][:, :], in_=seg_d[128:256, :])
            nc.sync.dma_start(out=dct_h[1][:, :], in_=dct_d[128:256, :])

            acc = psp.tile([3 * OC, 3 * NPTS], f32, tag="acc")
            outs = pool.tile([3 * OC, 3 * NPTS], f32, tag="outs")

            chunks = [(0, 0), (0, 1), (1, 0), (1, 1)]
            tiles = {}
            for idx in range(4):
                tiles[idx] = dict(
                    sqx=pool.tile([H, CH], b16, name=f"sqx{idx}", tag=f"sqx{idx}"),
                    sqy=pool.tile([H, CH], b16, name=f"sqy{idx}", tag=f"sqy{idx}"),
                    ew=pool.tile([H, CH], b16, name=f"ew{idx}", tag=f"ew{idx}"),
                    sp=pool.tile([H, CH], b16, name=f"sp{idx}", tag=f"sp{idx}"),
                    s=pool.tile([H, CH], f32, name=f"s{idx}", tag=f"s{idx}"),
                    rs=pool.tile([H, CH], f32, name=f"rs{idx}", tag=f"rs{idx}"),
                    u=pool.tile([H, CH], b16, name=f"u{idx}", tag=f"u{idx}"),
                    xy=pool.tile([H, CH], b16, name=f"xy{idx}", tag=f"xy{idx}"),
                    mx=pool.tile([H, 32], f32, name=f"mx{idx}", tag=f"mx{idx}"),
                    L=pool.tile([H, 3 * HC], b16, name=f"L{idx}", tag=f"L{idx}"),
                    R=pool.tile([H, 3 * CH], b16, name=f"R{idx}", tag=f"R{idx}"),
                )

            for idx, (hf, q) in enumerate(chunks):
                t = tiles[idx]
                seg_s = seg_h[hf][:, CH * q : CH * q + CH]
                dct_s = dct_h[hf][:, 2 * CH * q : 2 * CH * q + 2 * CH]
                w_s = w_h[hf][:, CH * q : CH * q + CH]
                dx = dct_s[:, 0::2]
                dy = dct_s[:, 1::2]

                # ---- ACT: squares + softplus (one resident table)
                nc.scalar.activation(out=t["sqx"][:, :], in_=dx, func=Act.Square)
                nc.scalar.activation(out=t["sqy"][:, :], in_=dy, func=Act.Square)
                nc.scalar.activation(out=t["ew"][:, :], in_=w_s, func=Act.Exp)
                nc.scalar.activation(
                    out=t["sp"][:, :], in_=t["ew"][:, :], func=Act.Ln, bias=1.0
                )

                # ---- Pool: per-pixel max, one-hot, x*y
                seg_wc = seg_s.rearrange("p (w c) -> p w c", c=NCLS)
                nc.vector.tensor_reduce(
                    out=t["mx"][:, :], in_=seg_wc, axis=Axis.X, op=Alu.max
                )
                seg_cw = seg_s.rearrange("p (w c) -> p c w", c=NCLS)[:, 1:NCLS, :]
                mx_b = t["mx"][:, :].unsqueeze(1).broadcast_to((H, OC, 32))
                hot = t["L"][:, 0:HC].rearrange("p (c w) -> p c w", c=OC)
                nc.vector.tensor_tensor(
                    out=hot, in0=seg_cw, in1=mx_b, op=Alu.is_equal
                )
                nc.gpsimd.tensor_tensor(
                    out=t["xy"][:, :], in0=dx, in1=dy, op=Alu.mult
                )

                # ---- ACT: hot * ch  (per-partition scale via Copy)
                nc.scalar.mul(
                    t["L"][:, HC : 2 * HC], t["L"][:, 0:HC], chv[:, hf : hf + 1]
                )

                # ---- DVE: s, 1/s, u, hot*cw, features
                nc.vector.tensor_tensor(
                    out=t["s"][:, :], in0=t["sqx"][:, :], in1=t["sqy"][:, :],
                    op=Alu.add,
                )
                nc.vector.reciprocal_approx_fast(t["rs"][:, :], t["s"][:, :])
                nc.vector.tensor_tensor(
                    out=t["u"][:, :], in0=t["rs"][:, :], in1=t["sp"][:, :],
                    op=Alu.mult,
                )
                hot_cw = t["L"][:, 2 * HC : 3 * HC].rearrange(
                    "p (c w) -> p c w", c=OC
                )
                cw_b = (
                    cw64[:, 32 * q : 32 * q + 32]
                    .unsqueeze(1)
                    .broadcast_to((H, OC, 32))
                )
                nc.gpsimd.tensor_tensor(
                    out=hot_cw, in0=hot, in1=cw_b, op=Alu.mult
                )
                nc.vector.tensor_tensor(
                    out=t["R"][:, 0:CH], in0=t["u"][:, :], in1=t["sqy"][:, :],
                    op=Alu.mult,
                )
                nc.vector.tensor_tensor(
                    out=t["R"][:, CH : 2 * CH], in0=t["u"][:, :],
                    in1=t["xy"][:, :], op=Alu.mult,
                )
                nc.vector.tensor_tensor(
                    out=t["R"][:, 2 * CH : 3 * CH], in0=t["u"][:, :],
                    in1=t["sqx"][:, :], op=Alu.mult,
                )

                # ---- PE: 32 accumulating matmuls (one per w-subcolumn)
                Lv = t["L"][:, :].rearrange("p (t w) -> p t w", w=32)
                Rv = t["R"][:, :].rearrange("p (b n) -> p b n", b=3)
                for j in range(32):
                    nc.tensor.matmul(
                        acc[:, :],
                        Lv[:, :, j],
                        Rv[:, :, NPTS * j : NPTS * j + NPTS],
                        start=(idx == 0 and j == 0),
                        stop=(idx == 3 and j == 31),
                    )

            nc.vector.tensor_copy(out=outs[:, :], in_=acc[:, :])
            nc.sync.dma_start(out=out_d[:, :], in_=outs[:, :])

    nc.compile()
    return nc


def _host_constants():
    import ml_dtypes

    bf16 = ml_dtypes.bfloat16
    pi = np.arange(128)
    chv = np.stack(
        [(64.0 * hf + pi // 2 + 0.5) / HEIGHT for hf in range(2)], axis=1
    ).astype(np.float32)
    j = np.arange(64)
    cw64 = (((pi % 2)[:, None] * 64 + j[None, :] + 0.5) / HEIGHT).astype(bf16)
    return chv, np.ascontiguousarray(cw64)


def _solve_host(acc_f32: np.ndarray) -> np.ndarray:
    """acc [24,27] fp32 -> p [OC, NPTS, 2] fp32 (float64 pinv like reference)."""
    a = acc_f32.astype(np.float64)
    A = a[0:OC, 0:9]
    Bm = a[0:OC, 9:18]
    D = a[0:OC, 18:27]
    S1 = a[OC : 2 * OC, 0:9]
    S3 = a[OC : 2 * OC, 9:18]
    S2 = a[2 * OC : 3 * OC, 9:18]
    S4 = a[2 * OC : 3 * OC, 18:27]
    Rm = np.empty((OC, NPTS, 2, 2), dtype=np.float64)
    Rm[..., 0, 0] = A
    Rm[..., 0, 1] = -Bm
    Rm[..., 1, 0] = -Bm
    Rm[..., 1, 1] = D
    q = np.stack([S1 - S2, S4 - S3], axis=-1)
    Rp = np.linalg.pinv(Rm.reshape(-1, 2, 2)).reshape(Rm.shape)
    p = np.einsum("cpij,cpj->cpi", Rp, q) * HEIGHT
    return p.astype(np.float32)


def kernel(seg, direct, w):
    if "nc" not in _cache:
        _cache["nc"] = _build_nc()
    nc = _cache["nc"]

    seg = np.ascontiguousarray(np.asarray(seg, dtype=np.float32))
    direct = np.ascontiguousarray(np.asarray(direct, dtype=np.float32))
    w = np.ascontiguousarray(np.asarray(w, dtype=np.float32))
    chv, cw64 = _host_constants()

    in_maps = []
    for i in range(B):
        in_maps.append(
            {
                "seg": seg[i].reshape(2 * H, W * NCLS // 2),
                "direct": direct[i].reshape(2 * H, W * NPTS),
                "w": w[i].reshape(2 * H, W * NPTS // 2),
                "chv": chv,
                "cw64": cw64,
            }
        )

    from concourse.bass_utils import run_bass_kernel_spmd

    trace = bool(int(os.environ.get("KERNEL_TRACE", "0")))
    res = run_bass_kernel_spmd(
        nc, in_maps, core_ids=list(range(N_CORES)), trace=trace
    )
    kernel._last_exec_ns = res.exec_time_ns
    kernel._last_results = res

    out = np.stack(
        [_solve_host(np.asarray(res.results[i]["acc"])) for i in range(B)], axis=0
    )
    return out
